# revision 24
# baseline (speedup 1.0000x reference)
"""Causal ReLU-attention (no softmax) fused kernel for TRN2, 8 NeuronCores.

Reference computation (B=2, T=2048, C=1024, H=16, D=64):
    qkv = x @ W.T + b ; q,k,v split; per (b,h): y = relu(tril(q k^T / sqrt(D))) @ v

Sharding: core c handles batch b = c//4 and heads 4*(c%4) .. 4*(c%4)+3.
Each core is fully independent (no collectives).

Per-core device kernel (all matmuls in float32r: full fp32 storage,
~1e-4 relative matmul error, 4x faster than float32 on the PE):
  phase 1: qT [256,2048], kT [256,2048] (head-major, scale folded into Wq),
           v [2048,256] from xT [1024,2048] and wT [1024,768].
  phase 2: per head h, per query-chunk qc (512 queries):
           ST block = K_blk @ Q_chunk^T -> [128 keys, 512 queries] in PSUM,
           ReLU-evacuate to SBUF (ACT/DVE split), triangular masks on the
           4 diagonal 128x128 tiles via batched affine_select,
           yT[64, 512] += V_blk^T-free matmul accumulation in PSUM.
Output per core: yT [256, 2048]; host transposes into y[b, :, 256g:256g+256].
"""

import numpy as np

N_EMBD = 1024
N_HEAD = 16
HEAD_DIM = 64
B, T, C = 2, 2048, N_EMBD
NCORES = 8
HPC = N_HEAD // 4  # heads per core = 4
P = 128
KC = C // P  # 8 contraction chunks
NQC = T // 512  # 4 query chunks
NKB = T // P  # 16 key blocks

_NC_CACHE = {}


def _build_bass():
    import concourse.bass as bass
    from concourse import bacc, mybir
    from concourse.tile import TileContext

    f32 = mybir.dt.float32
    f32r = mybir.dt.float32r
    bf16 = mybir.dt.bfloat16

    nc = bacc.Bacc(None, target_bir_lowering=False)
    xt = nc.declare_dram_parameter("xt", [C, T], f32, isOutput=False)
    wt = nc.declare_dram_parameter("wt", [C, 768], f32, isOutput=False)
    bcol = nc.declare_dram_parameter("bcol", [512], f32, isOutput=False)
    bv = nc.declare_dram_parameter("bv", [256], f32, isOutput=False)
    out = nc.declare_dram_parameter("out", [256, T], f32, isOutput=True)

    xt_r = xt[:, :].rearrange("(c p) t -> c p t", p=P)
    wt_r = wt[:, :].rearrange("(c p) o -> c p o", p=P)
    bv_ap = bv[:]

    with TileContext(nc) as tc:
        with (
            tc.tile_pool(name="const", bufs=1) as const_pool,
            tc.tile_pool(name="qkv", bufs=1) as qkv_pool,
            tc.tile_pool(name="stsb", bufs=4) as stsb_pool,
            tc.tile_pool(name="ysb", bufs=4) as ysb_pool,
            tc.tile_pool(name="psum1", bufs=2, space="PSUM") as psum1,
        ):
            # ---- constant / input loads ----
            xt_sb = const_pool.tile([P, KC, T], f32r)
            wt_sb = const_pool.tile([P, KC, 768], f32r)
            for c in range(KC):
                nc.sync.dma_start(out=xt_sb[:, c, :], in_=xt_r[c].bitcast(f32r))
                nc.sync.dma_start(out=wt_sb[:, c, :], in_=wt_r[c].bitcast(f32r))
            bqk_sb = const_pool.tile([P, 4], f32)
            nc.sync.dma_start(
                out=bqk_sb, in_=bcol[:].rearrange("(m p) -> p m", p=P)
            )
            # bv broadcast across partitions
            bv_rep = const_pool.tile([P, 256], f32)
            bv_bcast = bass.AP(tensor=bv_ap.tensor, offset=bv_ap.offset, ap=[[0, P], [1, 256]])
            nc.sync.dma_start(out=bv_rep, in_=bv_bcast)

            q_sb = qkv_pool.tile([P, 2, T], f32r)
            k_sb = qkv_pool.tile([P, 2, T], f32r)
            v_sb = qkv_pool.tile([P, NKB, 256], f32r)

            # ---- phase 1: qT / kT ----
            # q m0+m1 interleaved per contraction chunk: doubles the PE work
            # available per arriving xt chunk during the DMA-bound head.
            for dst_sb, wofs, bofs in ((q_sb, 0, 0), (k_sb, 256, 2)):
                pss = [
                    psum1.tile([P, T], f32, tag="ps1", name=f"ps1_{m}")
                    for m in range(2)
                ]
                for c in range(KC):
                    for m in range(2):
                        for n in range(NQC):
                            nc.tensor.matmul(
                                pss[m][:, n * 512 : (n + 1) * 512],
                                wt_sb[:, c, wofs + m * P : wofs + (m + 1) * P],
                                xt_sb[:, c, n * 512 : (n + 1) * 512],
                                start=(c == 0),
                                stop=(c == KC - 1),
                            )
                for m in range(2):
                    nc.vector.tensor_scalar_add(
                        dst_sb[:, m, :], pss[m], bqk_sb[:, bofs + m : bofs + m + 1]
                    )

            # ---- phase 1: v (natural layout [t, o]) ----
            for half in range(2):
                ps = psum1.tile([P, T], f32, tag="ps1")
                for tl in range(8):
                    tb = half * 8 + tl
                    for c in range(KC):
                        nc.tensor.matmul(
                            ps[:, tl * 256 : (tl + 1) * 256],
                            xt_sb[:, c, tb * P : (tb + 1) * P],
                            wt_sb[:, c, 512:768],
                            start=(c == 0),
                            stop=(c == KC - 1),
                        )
                v_dst = v_sb[:, half * 8 : (half + 1) * 8, :]
                bv_in = bass.AP(
                    tensor=bv_rep.tensor,
                    offset=bv_rep.offset,
                    ap=[bv_rep.ap[0], [0, 8], [1, 256]],
                )
                nc.vector.tensor_tensor(
                    out=v_dst, in0=ps.rearrange("p (a b) -> p a b", a=8), in1=bv_in,
                    op=mybir.AluOpType.add,
                )

        # ---- phase 2: attention, head PAIRS for PE concurrency ----
        # Head pair hp covers heads (2hp, 2hp+1), stored at SBUF partitions
        # [0:64] / [64:128] of q_sb/k_sb[:, hp, :].  The two heads' ST matmuls
        # use disjoint PE row-groups (tile row 0 vs 64) and run concurrently;
        # the AV matmuls use disjoint column groups writing yps[0:64]/[64:128].
        with (
            tc.tile_pool(name="stsb2", bufs=8) as stsb2_pool,
            tc.tile_pool(name="ysb2", bufs=4) as ysb2_pool,
            tc.tile_pool(name="pst", bufs=3, space="PSUM") as pst_pool,
            tc.tile_pool(name="py", bufs=1, space="PSUM") as py_pool,
        ):
            evac_ctr = 0
            for hp in range(2):
                for qc in range(NQC):
                    yps = [
                        py_pool.tile([64, 512], f32, tag=f"yps{i}", name=f"yps{i}")
                        for i in range(2)
                    ]
                    nblocks = 4 * (qc + 1)

                    def st_block(kb):
                        nonlocal evac_ctr
                        d = kb - 4 * qc
                        c0 = P * d if d > 0 else 0
                        stps = pst_pool.tile([P, 2, 512], f32, tag="stps", name="stps")
                        stsb = stsb2_pool.tile([P, 2, 512], f32r, tag="stsb", name="stsb")
                        for hh in range(2):
                            off = hh * 64
                            nc.tensor.matmul(
                                stps[:, hh, c0:512],
                                k_sb[off : off + 64, hp, kb * P : (kb + 1) * P],
                                q_sb[off : off + 64, hp, qc * 512 + c0 : (qc + 1) * 512],
                                start=True,
                                stop=True,
                            )
                        # relu evac: near the qc tail, split per-head across
                        # both engines (halves latency at the drain); else one
                        # op covering both heads, ACT-favored 3:2 alternation
                        if kb >= nblocks - 2:
                            nc.scalar.activation(
                                out=stsb[:, 0, c0:512],
                                in_=stps[:, 0, c0:512],
                                func=mybir.ActivationFunctionType.Relu,
                            )
                            nc.vector.tensor_scalar_max(
                                stsb[:, 1, c0:512], stps[:, 1, c0:512], 0.0
                            )
                        elif evac_ctr % 5 in (0, 2, 4):
                            nc.scalar.activation(
                                out=stsb[:, :, c0:512],
                                in_=stps[:, :, c0:512],
                                func=mybir.ActivationFunctionType.Relu,
                            )
                        else:
                            nc.vector.tensor_scalar_max(
                                stsb[:, :, c0:512], stps[:, :, c0:512], 0.0
                            )
                        evac_ctr += 1
                        if d >= 0:
                            # triangle mask, both heads in one strided op
                            base = stsb[:, 0, P * d : P * d + P]
                            tri = bass.AP(
                                tensor=base.tensor,
                                offset=base.offset,
                                ap=[base.ap[0], [512, 2], [1, P]],
                            )
                            nc.gpsimd.affine_select(
                                out=tri,
                                in_=tri,
                                pattern=[[0, 2], [1, P]],
                                compare_op=mybir.AluOpType.is_ge,
                                fill=0.0,
                                base=0,
                                channel_multiplier=-1,
                            )
                        return stsb

                    def av_block(kb, stsb):
                        d = kb - 4 * qc
                        c0 = P * d if d > 0 else 0
                        for hh in range(2):
                            h = 2 * hp + hh
                            nc.tensor.matmul(
                                yps[hh][:, c0:512],
                                v_sb[:, kb, h * 64 : (h + 1) * 64],
                                stsb[:, hh, c0:512],
                                start=(kb == 0),
                                stop=(kb == nblocks - 1),
                            )

                    # software-pipelined: AV(kb-SKEW) emitted after ST(kb) so
                    # the PE never waits on the ~1.7us evac latency
                    SKEW = 3
                    pending = []
                    for kb in range(nblocks):
                        pending.append((kb, st_block(kb)))
                        if len(pending) > SKEW:
                            pkb, pst = pending.pop(0)
                            av_block(pkb, pst)
                    for pkb, pst in pending:
                        av_block(pkb, pst)

                    for hh in range(2):
                        h = 2 * hp + hh
                        ysb = ysb2_pool.tile(
                            [64, 512], f32, tag=f"ysb{hh}", name=f"ysb{hh}"
                        )
                        if hh == 0:
                            nc.scalar.copy(ysb, yps[hh])
                        else:
                            nc.vector.tensor_copy(ysb, yps[hh])
                        nc.sync.dma_start(
                            out=out[h * 64 : (h + 1) * 64, qc * 512 : (qc + 1) * 512],
                            in_=ysb,
                        )

    nc.compile()
    return nc


def _get_nc():
    if "nc" not in _NC_CACHE:
        _NC_CACHE["nc"] = _build_bass()
    return _NC_CACHE["nc"]


def kernel(x: np.ndarray, W: np.ndarray, b: np.ndarray) -> np.ndarray:
    from concourse.bass_utils import run_bass_kernel_spmd

    x = np.asarray(x, dtype=np.float32)
    W = np.asarray(W, dtype=np.float32)
    b = np.asarray(b, dtype=np.float32)

    nc = _get_nc()
    scale = np.float32(1.0 / np.sqrt(HEAD_DIM))

    xts = [np.ascontiguousarray(x[bb].T) for bb in range(B)]
    in_maps = []
    for core in range(NCORES):
        bb, g = core // 4, core % 4
        o0 = g * 256
        wq = W[o0 : o0 + 256, :] * scale
        wk = W[C + o0 : C + o0 + 256, :]
        wv = W[2 * C + o0 : 2 * C + o0 + 256, :]
        wt = np.ascontiguousarray(
            np.concatenate([wq.T, wk.T, wv.T], axis=1), dtype=np.float32
        )
        bq = b[o0 : o0 + 256] * scale
        bk = b[C + o0 : C + o0 + 256]
        bv = np.ascontiguousarray(b[2 * C + o0 : 2 * C + o0 + 256], dtype=np.float32)
        bcol = np.ascontiguousarray(np.concatenate([bq, bk]), dtype=np.float32)
        in_maps.append({"xt": xts[bb], "wt": wt, "bcol": bcol, "bv": bv})

    res = run_bass_kernel_spmd(nc, in_maps, core_ids=list(range(NCORES)))

    y = np.empty((B, T, C), dtype=np.float32)
    for core in range(NCORES):
        bb, g = core // 4, core % 4
        y[bb, :, g * 256 : (g + 1) * 256] = res.results[core]["out"].T
    return y


# revision 25
# speedup vs baseline: 1.1890x; 1.1890x over previous
"""Causal ReLU-attention (no softmax) fused kernel for TRN2, 8 NeuronCores.

Reference computation (B=2, T=2048, C=1024, H=16, D=64):
    qkv = x @ W.T + b ; q,k,v split; per (b,h): y = relu(tril(q k^T / sqrt(D))) @ v

Sharding: core c handles batch b = c//4 and heads 4*(c%4) .. 4*(c%4)+3.
Each core is fully independent (no collectives).

Per-core device kernel (all matmuls in float32r: full fp32 storage,
~1e-4 relative matmul error, 4x faster than float32 on the PE):
  phase 1: qT [256,2048], kT [256,2048] (head-major, scale folded into Wq),
           v [2048,256] from xT [1024,2048] and wT [1024,768].
  phase 2: per head h, per query-chunk qc (512 queries):
           ST block = K_blk @ Q_chunk^T -> [128 keys, 512 queries] in PSUM,
           ReLU-evacuate to SBUF (ACT/DVE split), triangular masks on the
           4 diagonal 128x128 tiles via batched affine_select,
           yT[64, 512] += V_blk^T-free matmul accumulation in PSUM.
Output per core: yT [256, 2048]; host transposes into y[b, :, 256g:256g+256].
"""

import numpy as np

N_EMBD = 1024
N_HEAD = 16
HEAD_DIM = 64
B, T, C = 2, 2048, N_EMBD
NCORES = 8
HPC = N_HEAD // 4  # heads per core = 4
P = 128
KC = C // P  # 8 contraction chunks
NQC = T // 512  # 4 query chunks
NKB = T // P  # 16 key blocks

_NC_CACHE = {}


def _build_bass():
    import concourse.bass as bass
    from concourse import bacc, mybir
    from concourse.tile import TileContext

    f32 = mybir.dt.float32
    f32r = mybir.dt.float32r
    bf16 = mybir.dt.bfloat16

    nc = bacc.Bacc(None, target_bir_lowering=False)
    xt = nc.declare_dram_parameter("xt", [C, T], f32, isOutput=False)
    wt = nc.declare_dram_parameter("wt", [C, 768], f32, isOutput=False)
    bcol = nc.declare_dram_parameter("bcol", [512], f32, isOutput=False)
    bv = nc.declare_dram_parameter("bv", [256], f32, isOutput=False)
    out = nc.declare_dram_parameter("out", [256, T], f32, isOutput=True)

    xt_r = xt[:, :].rearrange("(c p) t -> c p t", p=P)
    wt_r = wt[:, :].rearrange("(c p) o -> c p o", p=P)
    bv_ap = bv[:]

    with TileContext(nc) as tc:
        with (
            tc.tile_pool(name="const", bufs=1) as const_pool,
            tc.tile_pool(name="qkv", bufs=1) as qkv_pool,
            tc.tile_pool(name="stsb", bufs=4) as stsb_pool,
            tc.tile_pool(name="ysb", bufs=4) as ysb_pool,
            tc.tile_pool(name="psum1", bufs=2, space="PSUM") as psum1,
        ):
            # ---- constant / input loads ----
            xt_sb = const_pool.tile([P, KC, T], f32r)
            wt_sb = const_pool.tile([P, KC, 768], f32r)
            for c in range(KC):
                nc.sync.dma_start(out=xt_sb[:, c, :], in_=xt_r[c].bitcast(f32r))
                nc.sync.dma_start(out=wt_sb[:, c, :], in_=wt_r[c].bitcast(f32r))
            bqk_sb = const_pool.tile([P, 4], f32)
            nc.sync.dma_start(
                out=bqk_sb, in_=bcol[:].rearrange("(m p) -> p m", p=P)
            )
            # bv broadcast across partitions
            bv_rep = const_pool.tile([P, 256], f32)
            bv_bcast = bass.AP(tensor=bv_ap.tensor, offset=bv_ap.offset, ap=[[0, P], [1, 256]])
            nc.sync.dma_start(out=bv_rep, in_=bv_bcast)

            q_sb = qkv_pool.tile([P, 2, T], f32r)
            k_sb = qkv_pool.tile([P, 2, T], f32r)
            v_sb = qkv_pool.tile([P, NKB, 256], f32r)

            # ---- phase 1: qT / kT ----
            # q m0+m1 interleaved per contraction chunk: doubles the PE work
            # available per arriving xt chunk during the DMA-bound head.
            for dst_sb, wofs, bofs in ((q_sb, 0, 0), (k_sb, 256, 2)):
                pss = [
                    psum1.tile([P, T], f32, tag="ps1", name=f"ps1_{m}")
                    for m in range(2)
                ]
                for c in range(KC):
                    for m in range(2):
                        for n in range(NQC):
                            nc.tensor.matmul(
                                pss[m][:, n * 512 : (n + 1) * 512],
                                wt_sb[:, c, wofs + m * P : wofs + (m + 1) * P],
                                xt_sb[:, c, n * 512 : (n + 1) * 512],
                                start=(c == 0),
                                stop=(c == KC - 1),
                            )
                for m in range(2):
                    nc.vector.tensor_scalar_add(
                        dst_sb[:, m, :], pss[m], bqk_sb[:, bofs + m : bofs + m + 1]
                    )

            # ---- phase 1: v (natural layout [t, o]) ----
            for half in range(2):
                ps = psum1.tile([P, T], f32, tag="ps1")
                for tl in range(8):
                    tb = half * 8 + tl
                    for c in range(KC):
                        nc.tensor.matmul(
                            ps[:, tl * 256 : (tl + 1) * 256],
                            xt_sb[:, c, tb * P : (tb + 1) * P],
                            wt_sb[:, c, 512:768],
                            start=(c == 0),
                            stop=(c == KC - 1),
                        )
                v_dst = v_sb[:, half * 8 : (half + 1) * 8, :]
                bv_in = bass.AP(
                    tensor=bv_rep.tensor,
                    offset=bv_rep.offset,
                    ap=[bv_rep.ap[0], [0, 8], [1, 256]],
                )
                nc.vector.tensor_tensor(
                    out=v_dst, in0=ps.rearrange("p (a b) -> p a b", a=8), in1=bv_in,
                    op=mybir.AluOpType.add,
                )

        # ---- phase 2: attention, head PAIRS for PE concurrency ----
        # Head pair hp covers heads (2hp, 2hp+1), stored at SBUF partitions
        # [0:64] / [64:128] of q_sb/k_sb[:, hp, :].  The two heads' ST matmuls
        # use disjoint PE row-groups (tile row 0 vs 64) and run concurrently;
        # the AV matmuls use disjoint column groups writing yps[0:64]/[64:128].
        with (
            tc.tile_pool(name="stsb2", bufs=8) as stsb2_pool,
            tc.tile_pool(name="ysb2", bufs=4) as ysb2_pool,
            tc.tile_pool(name="pst", bufs=3, space="PSUM") as pst_pool,
            tc.tile_pool(name="py", bufs=1, space="PSUM") as py_pool,
        ):
            evac_ctr = 0
            for hp in range(2):
                for qc in range(NQC):
                    yps = [
                        py_pool.tile([64, 512], f32, tag=f"yps{i}", name=f"yps{i}")
                        for i in range(2)
                    ]
                    nblocks = 4 * (qc + 1)

                    def st_block(kb):
                        nonlocal evac_ctr
                        d = kb - 4 * qc
                        c0 = P * d if d > 0 else 0
                        stps = pst_pool.tile([P, 2, 512], f32, tag="stps", name="stps")
                        stsb = stsb2_pool.tile([P, 2, 512], f32r, tag="stsb", name="stsb")
                        for hh in range(2):
                            off = hh * 64
                            nc.tensor.matmul(
                                stps[:, hh, c0:512],
                                k_sb[off : off + 64, hp, kb * P : (kb + 1) * P],
                                q_sb[off : off + 64, hp, qc * 512 + c0 : (qc + 1) * 512],
                                start=True,
                                stop=True,
                            )
                        # relu evac, one op covering both heads, alternating engine
                        if evac_ctr % 2 == 0:
                            nc.scalar.activation(
                                out=stsb[:, :, c0:512],
                                in_=stps[:, :, c0:512],
                                func=mybir.ActivationFunctionType.Relu,
                            )
                        else:
                            nc.vector.tensor_scalar_max(
                                stsb[:, :, c0:512], stps[:, :, c0:512], 0.0
                            )
                        evac_ctr += 1
                        if d >= 0:
                            # triangle mask, both heads in one strided op
                            base = stsb[:, 0, P * d : P * d + P]
                            tri = bass.AP(
                                tensor=base.tensor,
                                offset=base.offset,
                                ap=[base.ap[0], [512, 2], [1, P]],
                            )
                            nc.gpsimd.affine_select(
                                out=tri,
                                in_=tri,
                                pattern=[[0, 2], [1, P]],
                                compare_op=mybir.AluOpType.is_ge,
                                fill=0.0,
                                base=0,
                                channel_multiplier=-1,
                            )
                        return stsb

                    def av_block(kb, stsb):
                        d = kb - 4 * qc
                        c0 = P * d if d > 0 else 0
                        for hh in range(2):
                            h = 2 * hp + hh
                            nc.tensor.matmul(
                                yps[hh][:, c0:512],
                                v_sb[:, kb, h * 64 : (h + 1) * 64],
                                stsb[:, hh, c0:512],
                                start=(kb == 0),
                                stop=(kb == nblocks - 1),
                            )

                    # software-pipelined: AV(kb-SKEW) emitted after ST(kb) so
                    # the PE never waits on the ~1.7us evac latency
                    SKEW = 3
                    pending = []
                    for kb in range(nblocks):
                        pending.append((kb, st_block(kb)))
                        if len(pending) > SKEW:
                            pkb, pst = pending.pop(0)
                            av_block(pkb, pst)
                    for pkb, pst in pending:
                        av_block(pkb, pst)

                    for hh in range(2):
                        h = 2 * hp + hh
                        ysb = ysb2_pool.tile(
                            [64, 512], f32, tag=f"ysb{hh}", name=f"ysb{hh}"
                        )
                        if hh == 0:
                            nc.scalar.copy(ysb, yps[hh])
                        else:
                            nc.vector.tensor_copy(ysb, yps[hh])
                        nc.sync.dma_start(
                            out=out[h * 64 : (h + 1) * 64, qc * 512 : (qc + 1) * 512],
                            in_=ysb,
                        )

    nc.compile()
    return nc


def _get_nc():
    if "nc" not in _NC_CACHE:
        _NC_CACHE["nc"] = _build_bass()
    return _NC_CACHE["nc"]


def kernel(x: np.ndarray, W: np.ndarray, b: np.ndarray) -> np.ndarray:
    from concourse.bass_utils import run_bass_kernel_spmd

    x = np.asarray(x, dtype=np.float32)
    W = np.asarray(W, dtype=np.float32)
    b = np.asarray(b, dtype=np.float32)

    nc = _get_nc()
    scale = np.float32(1.0 / np.sqrt(HEAD_DIM))

    xts = [np.ascontiguousarray(x[bb].T) for bb in range(B)]
    in_maps = []
    for core in range(NCORES):
        bb, g = core // 4, core % 4
        o0 = g * 256
        wq = W[o0 : o0 + 256, :] * scale
        wk = W[C + o0 : C + o0 + 256, :]
        wv = W[2 * C + o0 : 2 * C + o0 + 256, :]
        wt = np.ascontiguousarray(
            np.concatenate([wq.T, wk.T, wv.T], axis=1), dtype=np.float32
        )
        bq = b[o0 : o0 + 256] * scale
        bk = b[C + o0 : C + o0 + 256]
        bv = np.ascontiguousarray(b[2 * C + o0 : 2 * C + o0 + 256], dtype=np.float32)
        bcol = np.ascontiguousarray(np.concatenate([bq, bk]), dtype=np.float32)
        in_maps.append({"xt": xts[bb], "wt": wt, "bcol": bcol, "bv": bv})

    res = run_bass_kernel_spmd(nc, in_maps, core_ids=list(range(NCORES)))

    y = np.empty((B, T, C), dtype=np.float32)
    for core in range(NCORES):
        bb, g = core // 4, core % 4
        y[bb, :, g * 256 : (g + 1) * 256] = res.results[core]["out"].T
    return y
